# revision 10
# baseline (speedup 1.0000x reference)
"""Conv1DBlockSqueezeformer fused Bass kernel for 8 TRN2 NeuronCores.

Data-parallel over batch: 32 batches -> 4 per core. Everything on-chip is
feature-major ([channel partitions, time free]) so the depthwise time-conv,
BN/SiLU, ECA and both GLU-MLPs chain without transposes; x is transposed
to [B, C, T] on the host.

Matmul dtypes: fp32r (full-rate fp32) for expand/ffn1/stats and the
residual spine; bf16 for the depthwise conv (block-diagonal matmuls),
projection and ffn2. The depthwise conv runs on the TensorEngine as 17
PSUM-accumulated matmuls with diagonal [128,128] weight blocks (one per
tap), streamed from HBM.
"""
import os
import sys

os.environ.setdefault("MYCRO_LOCAL_CACHE", "1")
sys.path.insert(0, "/opt/trn_rl_repo")

import numpy as np
import ml_dtypes

B, T, C = 32, 2048, 384
KTAP = 17
PAD = KTAP // 2          # 8
E = 4 * C                # 1536
H = E // 2               # 768
NCORES = 8
BPC = B // NCORES        # 4 batches per core
NH = T // 2              # half-length 1024
BN_EPS = 1e-5
LN_EPS = 1e-6

_CACHE = {}


def _build():
    import concourse.bacc as bacc
    import concourse.mybir as mybir
    import concourse.tile as tile

    dt = mybir.dt
    AF = mybir.ActivationFunctionType
    OP = mybir.AluOpType
    f32, f32r, bf16 = dt.float32, dt.float32r, dt.bfloat16

    nc = bacc.Bacc(None)
    P = nc.declare_dram_parameter

    xt_e = P("xt", [BPC, C, T], f32, isOutput=False)
    w1_e = P("w1", [128, 3, E], f32, isOutput=False)
    w2_e = P("w2", [128, 3, E], f32, isOutput=False)
    w3_e = P("w3", [128, 6, C], bf16, isOutput=False)
    pw_e = P("pw", [128, 6, C], bf16, isOutput=False)
    dwd_e = P("dwd", [3, 2, 128, KTAP, 128], bf16, isOutput=False)
    band_e = P("band", [128, 16, 128], bf16, isOutput=False)
    b1a_e = P("b1a", [128, 6], f32, isOutput=False)
    b1g_e = P("b1g", [128, 6], f32, isOutput=False)
    b2a_e = P("b2a", [128, 6], f32, isOutput=False)
    b2g_e = P("b2g", [128, 6], f32, isOutput=False)
    bns_e = P("bns", [128, 6], f32, isOutput=False)
    bnb_e = P("bnb", [128, 6], f32, isOutput=False)
    s1_e = P("s1", [128, 3], f32, isOutput=False)
    bb1_e = P("bb1", [128, 3], f32, isOutput=False)
    s2_e = P("s2", [128, 3], f32, isOutput=False)
    b2r_e = P("b2r", [1, C], bf16, isOutput=False)
    lng_e = P("lng", [128, 3], f32, isOutput=False)
    lnb_e = P("lnb", [128, 3], f32, isOutput=False)
    ones_e = P("onesc", [128, 1], f32, isOutput=False)
    out_e = P("out", [BPC, C, T], f32, isOutput=True)

    with tile.TileContext(nc) as tc:
        with tc.tile_pool(name="wts", bufs=1) as wts, \
             tc.tile_pool(name="xr", bufs=2) as xrp, \
             tc.tile_pool(name="hf", bufs=2) as hfp, \
             tc.tile_pool(name="h2", bufs=1) as h2p, \
             tc.tile_pool(name="r1", bufs=2) as r1p, \
             tc.tile_pool(name="yy", bufs=1) as yyp, \
             tc.tile_pool(name="sm", bufs=2) as smp, \
             tc.tile_pool(name="dw", bufs=2) as dwp, \
             tc.tile_pool(name="bc", bufs=1) as bcp, \
             tc.tile_pool(name="ob", bufs=2) as obp, \
             tc.tile_pool(name="u1", bufs=1) as u1p, \
             tc.tile_pool(name="psA", bufs=2, space="PSUM") as psA, \
             tc.tile_pool(name="psG", bufs=2, space="PSUM") as psG, \
             tc.tile_pool(name="psC", bufs=2, space="PSUM") as psC, \
             tc.tile_pool(name="psM", bufs=2, space="PSUM") as psM:

            # ---- preamble: weights ----
            w1 = wts.tile([128, 3, E], f32r, tag="w1")
            nc.sync.dma_start(w1[:], w1_e[:].bitcast(f32r))
            w2 = wts.tile([128, 3, E], f32r, tag="w2")
            nc.sync.dma_start(w2[:], w2_e[:].bitcast(f32r))
            w3 = wts.tile([128, 6, C], bf16, tag="w3")
            nc.sync.dma_start(w3[:], w3_e[:])
            pw = wts.tile([128, 6, C], bf16, tag="pw")
            nc.sync.dma_start(pw[:], pw_e[:])
            band = wts.tile([128, 16, 128], bf16, tag="band")
            nc.sync.dma_start(band[:], band_e[:])
            vec = {}
            for nm, ext, k in [("b1a", b1a_e, 6), ("b1g", b1g_e, 6),
                               ("b2a", b2a_e, 6), ("b2g", b2g_e, 6),
                               ("bns", bns_e, 6), ("bnb", bnb_e, 6),
                               ("s1", s1_e, 3), ("bb1", bb1_e, 3),
                               ("s2", s2_e, 3), ("lng", lng_e, 3),
                               ("lnb", lnb_e, 3)]:
                t = wts.tile([128, k], f32, tag=nm)
                nc.sync.dma_start(t[:], ext[:])
                vec[nm] = t
            ones_r = wts.tile([128, 1], f32r, tag="onr")
            nc.sync.dma_start(ones_r[:], ones_e[:].bitcast(f32r))
            ones5 = wts.tile([1, 512], bf16, tag="on5")
            nc.gpsimd.memset(ones5[:], 1.0)
            b2row = wts.tile([1, C], bf16, tag="b2r")
            nc.sync.dma_start(b2row[:], b2r_e[:])
            epsap = wts.tile([1, 1], f32, tag="eps")
            nc.gpsimd.memset(epsap[:], LN_EPS)

            for b in range(BPC):
                # ---- x load (fp32r direct) ----
                xr = []
                for hh in range(2):
                    xt = xrp.tile([128, 3, NH], f32r, tag="xr")
                    src = xt_e[b].rearrange("(a p) t -> p a t", p=128)
                    nc.sync.dma_start(xt[:], src[:, :, hh * NH:(hh + 1) * NH].bitcast(f32r))
                    xr.append(xt)

                # ---- expand + GLU -> h (bf16, with 8-col halos) ----
                hts = []
                for hh in range(2):
                    ht = hfp.tile([128, 6, NH + 16], bf16, tag="hf")
                    nc.gpsimd.memset(ht[:, :, 0:8], 0.0)
                    nc.gpsimd.memset(ht[:, :, NH + 8:NH + 16], 0.0)
                    hts.append(ht)
                for ch in range(4):
                    hh, off = ch // 2, (ch % 2) * 512
                    for m in range(6):
                        pg = psG.tile([128, 512], f32, tag="g")
                        for c in range(3):
                            nc.tensor.matmul(pg[:], w1[:, c, 128 * (m + 6):128 * (m + 7)],
                                             xr[hh][:, c, off:off + 512],
                                             start=(c == 0), stop=(c == 2))
                        pa = psA.tile([128, 512], f32, tag="a")
                        for c in range(3):
                            nc.tensor.matmul(pa[:], w1[:, c, 128 * m:128 * (m + 1)],
                                             xr[hh][:, c, off:off + 512],
                                             start=(c == 0), stop=(c == 2))
                        sg = smp.tile([128, 512], bf16, tag="sg")
                        nc.scalar.activation(sg[:], pg[:], AF.Silu,
                                             bias=vec["b1g"][:, m:m + 1], scale=1.0)
                        nc.vector.scalar_tensor_tensor(
                            hts[hh][:, m, 8 + off:8 + off + 512], pa[:],
                            vec["b1a"][:, m:m + 1], sg[:],
                            op0=OP.add, op1=OP.mult)
                # halo exchange between halves
                nc.vector.tensor_copy(hts[0][:, :, NH + 8:NH + 16], hts[1][:, :, 8:16])
                nc.vector.tensor_copy(hts[1][:, :, 0:8], hts[0][:, :, NH:NH + 8])

                # ---- depthwise conv (PE diag matmuls) + BN + SiLU -> h2 ----
                h2 = h2p.tile([128, 6, T], bf16, tag="h2")
                poolp = smp.tile([128, 6, 4], f32, tag="pool")
                for ch in range(4):
                    hh, off = ch // 2, (ch % 2) * 512
                    for w in range(3):
                        dts = []
                        for qq in range(2):
                            dwt = dwp.tile([128, KTAP, 128], bf16, tag="dww",
                                           name=f"dw{b}_{ch}_{w}_{qq}")
                            nc.sync.dma_start(dwt[:], dwd_e[w, qq])
                            dts.append(dwt)
                        pcs = [psC.tile([128, 512], f32, tag="c", name=f"pc{ch}_{w}_{i}")
                               for i in range(2)]
                        for k in range(KTAP):
                            for qq in range(2):
                                q = 2 * w + qq
                                nc.tensor.matmul(
                                    pcs[qq][:], dts[qq][:, k, :],
                                    hts[hh][:, q, off + k:off + k + 512],
                                    start=(k == 0), stop=(k == KTAP - 1),
                                    skip_group_check=True)
                        for qq in range(2):
                            q = 2 * w + qq
                            nc.scalar.activation(
                                h2[:, q, ch * 512:(ch + 1) * 512], pcs[qq][:], AF.Silu,
                                bias=vec["bnb"][:, q:q + 1], scale=vec["bns"][:, q:q + 1],
                                accum_out=poolp[:, q, ch:ch + 1])

                # ---- ECA gate + fold into proj weights ----
                pe1 = smp.tile([128, 6], f32, tag="pe1")
                pe2 = smp.tile([128, 6], f32, tag="pe2")
                pool = smp.tile([128, 6], f32, tag="pe3")
                nc.vector.tensor_add(pe1[:], poolp[:, :, 0], poolp[:, :, 1])
                nc.vector.tensor_add(pe2[:], poolp[:, :, 2], poolp[:, :, 3])
                nc.vector.tensor_add(pool[:], pe1[:], pe2[:])
                pool_bf = smp.tile([128, 6], bf16, tag="pe4")
                nc.vector.tensor_copy(pool_bf[:], pool[:])
                gps = psM.tile([128, 512], f32, tag="m")
                bi = 0
                for q2 in range(6):
                    qs = [q for q in (q2 - 1, q2, q2 + 1) if 0 <= q < 6]
                    for i, q in enumerate(qs):
                        nc.tensor.matmul(gps[:, q2:q2 + 1],
                                         band[:, bi, :],
                                         pool_bf[:, q:q + 1],
                                         start=(i == 0), stop=(i == len(qs) - 1),
                                         skip_group_check=True)
                        bi += 1
                gate = smp.tile([128, 6], f32, tag="gate")
                nc.scalar.activation(gate[:], gps[:, 0:6], AF.Sigmoid, bias=0.0, scale=1.0)
                pwg = u1p.tile([128, 6, C], bf16, tag="pwg")
                for q in range(6):
                    nc.vector.tensor_scalar_mul(pwg[:, q, :], pw[:, q, :], gate[:, q:q + 1])

                # ---- proj + scale/bias + residual -> r1 (f32r) ----
                r1 = []
                for hh in range(2):
                    r1.append(r1p.tile([128, 3, NH], f32r, tag="r1", name=f"r1_{b}_{hh}"))
                for ch in range(4):
                    hh, off = ch // 2, (ch % 2) * 512
                    for m in range(3):
                        pp = psM.tile([128, 512], f32, tag="m")
                        for q in range(6):
                            nc.tensor.matmul(pp[:], pwg[:, q, 128 * m:128 * (m + 1)],
                                             h2[:, q, ch * 512:(ch + 1) * 512],
                                             start=(q == 0), stop=(q == 5))
                        pt = smp.tile([128, 512], f32, tag="t1")
                        nc.scalar.activation(pt[:], pp[:], AF.Identity,
                                             bias=vec["bb1"][:, m:m + 1],
                                             scale=vec["s1"][:, m:m + 1])
                        nc.vector.tensor_add(r1[hh][:, m, off:off + 512], pt[:],
                                             xr[hh][:, m, off:off + 512])

                # ---- ffn1 + GLU -> f (bf16) ----
                fts = []
                for hh in range(2):
                    fts.append(hfp.tile([128, 6, NH + 16], bf16, tag="hf", name=f"f_{b}_{hh}"))
                for ch in range(4):
                    hh, off = ch // 2, (ch % 2) * 512
                    for m in range(6):
                        pg = psG.tile([128, 512], f32, tag="g")
                        for c in range(3):
                            nc.tensor.matmul(pg[:], w2[:, c, 128 * (m + 6):128 * (m + 7)],
                                             r1[hh][:, c, off:off + 512],
                                             start=(c == 0), stop=(c == 2))
                        pa = psA.tile([128, 512], f32, tag="a")
                        for c in range(3):
                            nc.tensor.matmul(pa[:], w2[:, c, 128 * m:128 * (m + 1)],
                                             r1[hh][:, c, off:off + 512],
                                             start=(c == 0), stop=(c == 2))
                        sg = smp.tile([128, 512], bf16, tag="sg")
                        nc.scalar.activation(sg[:], pg[:], AF.Silu,
                                             bias=vec["b2g"][:, m:m + 1], scale=1.0)
                        nc.vector.scalar_tensor_tensor(
                            fts[hh][:, m, 8 + off:8 + off + 512], pa[:],
                            vec["b2a"][:, m:m + 1], sg[:],
                            op0=OP.add, op1=OP.mult)

                # ---- ffn2 + sb2 + residual -> y; LN stats; LN apply ----
                for hh in range(2):
                    yt = yyp.tile([128, 3, NH], f32r, tag="y", name=f"y_{b}_{hh}")
                    rows_mu = bcp.tile([128, NH], f32, tag="rowm", name=f"rm_{b}_{hh}")
                    rows_sq = bcp.tile([128, NH], f32, tag="rowq", name=f"rq_{b}_{hh}")
                    rr_t = bcp.tile([128, NH], f32, tag="rowr", name=f"rx_{b}_{hh}")
                    for cc in range(2):
                        off = cc * 512
                        for m in range(3):
                            pf = psM.tile([128, 512], f32, tag="m")
                            for q in range(6):
                                nc.tensor.matmul(pf[:], w3[:, q, 128 * m:128 * (m + 1)],
                                                 fts[hh][:, q, 8 + off:8 + off + 512],
                                                 start=(q == 0), stop=False,
                                                 skip_group_check=True)
                            nc.tensor.matmul(pf[:], b2row[:, 128 * m:128 * (m + 1)],
                                             ones5[:], start=False, stop=True,
                                             skip_group_check=True)
                            nc.vector.scalar_tensor_tensor(
                                yt[:, m, off:off + 512], pf[:], vec["s2"][:, m:m + 1],
                                r1[hh][:, m, off:off + 512],
                                op0=OP.mult, op1=OP.add)
                        pmu = psM.tile([128, 512], f32, tag="m")
                        for m in range(3):
                            nc.tensor.matmul(pmu[0:1, :], ones_r[:],
                                             yt[:, m, off:off + 512],
                                             start=(m == 0), stop=(m == 2))
                        nc.scalar.copy(rows_mu[0:1, off:off + 512], pmu[0:1, :])
                        psq = psM.tile([128, 512], f32, tag="m")
                        for m in range(3):
                            y2m = smp.tile([128, 512], f32r, tag="t2")
                            nc.scalar.activation(y2m[:], yt[:, m, off:off + 512],
                                                 AF.Square, bias=0.0, scale=1.0)
                            nc.tensor.matmul(psq[0:1, :], ones_r[:], y2m[:],
                                             start=(m == 0), stop=(m == 2))
                        nc.scalar.copy(rows_sq[0:1, off:off + 512], psq[0:1, :])
                    # LN row math on partition 0, per chunk
                    for cc in range(2):
                        off = cc * 512
                        m2t = smp.tile([128, 512], f32, tag="t1")
                        nc.scalar.activation(m2t[0:1, :], rows_mu[0:1, off:off + 512],
                                             AF.Square, bias=0.0, scale=1.0 / C)
                        vvt = smp.tile([128, 512], f32, tag="t2")
                        nc.vector.scalar_tensor_tensor(
                            vvt[0:1, :], rows_sq[0:1, off:off + 512], 1.0 / C,
                            m2t[0:1, :], op0=OP.mult, op1=OP.subtract)
                        nc.scalar.activation(rr_t[0:1, off:off + 512], vvt[0:1, :],
                                             AF.Abs_reciprocal_sqrt, bias=epsap[:], scale=1.0)
                        nc.vector.scalar_tensor_tensor(
                            rows_sq[0:1, off:off + 512], rows_mu[0:1, off:off + 512],
                            1.0 / C, rr_t[0:1, off:off + 512],
                            op0=OP.mult, op1=OP.mult)
                    rb = bcp.tile([128, NH], f32, tag="rb", name=f"rbb_{b}_{hh}")
                    nc.gpsimd.partition_broadcast(rb[:], rr_t[0:1, :])
                    pb = bcp.tile([128, NH], f32, tag="pb", name=f"pbb_{b}_{hh}")
                    nc.gpsimd.partition_broadcast(pb[:], rows_sq[0:1, :])
                    dst = out_e[b].rearrange("(a p) t -> p a t", p=128)
                    for cc in range(2):
                        off = cc * 512
                        t0g = hh * NH + off
                        for m in range(3):
                            t1 = smp.tile([128, 512], f32, tag="t1")
                            nc.vector.tensor_mul(t1[:], yt[:, m, off:off + 512],
                                                 rb[:, off:off + 512])
                            t2 = smp.tile([128, 512], f32, tag="t2")
                            nc.vector.tensor_sub(t2[:], t1[:], pb[:, off:off + 512])
                            osb = obp.tile([128, 512], f32, tag="osb",
                                           name=f"osb_{b}_{hh}_{cc}_{m}")
                            nc.scalar.activation(osb[:], t2[:],
                                                 AF.Identity, bias=vec["lnb"][:, m:m + 1],
                                                 scale=vec["lng"][:, m:m + 1])
                            nc.sync.dma_start(dst[:, m, t0g:t0g + 512], osb[:])

    nc.compile()
    return nc


def _host_arrays(inputs):
    f32 = np.float32
    bf16 = ml_dtypes.bfloat16
    g = {k: np.asarray(v) for k, v in inputs.items()}

    xt = np.ascontiguousarray(np.asarray(g["x"], f32).transpose(0, 2, 1))  # [B, C, T]

    def ptile(v, k):
        return np.ascontiguousarray(np.asarray(v, f32).reshape(k, 128).T)  # [128, k]

    w1 = np.ascontiguousarray(
        np.asarray(g["expand_w"], f32).reshape(3, 128, E).transpose(1, 0, 2))
    w2 = np.ascontiguousarray(
        np.asarray(g["ffn_w1"], f32).reshape(3, 128, E).transpose(1, 0, 2))
    w3 = np.ascontiguousarray(
        np.asarray(g["ffn_w2"], f32).reshape(6, 128, C).transpose(1, 0, 2)).astype(bf16)
    pwa = np.ascontiguousarray(
        np.asarray(g["proj_w"], f32).reshape(6, 128, C).transpose(1, 0, 2)).astype(bf16)

    dw = np.asarray(g["dw_w"], f32)[:, 0, :]          # [H, K]
    dwd = np.zeros((3, 2, 128, KTAP, 128), f32)
    for w in range(3):
        for qq in range(2):
            q = 2 * w + qq
            blk = dw[128 * q:128 * (q + 1), :]        # [128, K]
            for k in range(KTAP):
                dwd[w, qq, :, k, :][np.arange(128), np.arange(128)] = blk[:, k]
    dwd = dwd.astype(bf16)

    ew = np.asarray(g["eca_w"], f32)[0, 0, :]         # [5]
    bandf = np.zeros((H, H), f32)
    for k in range(5):
        cp = np.arange(H)
        src = cp + k - 2
        m = (src >= 0) & (src < H)
        bandf[src[m], cp[m]] += ew[k]
    bandf /= float(T)
    blocks = []
    for q2 in range(6):
        for q in (q2 - 1, q2, q2 + 1):
            if 0 <= q < 6:
                blocks.append(bandf[128 * q:128 * (q + 1), 128 * q2:128 * (q2 + 1)])
    band = np.ascontiguousarray(np.stack(blocks, axis=1)).astype(bf16)  # [128, 16, 128]

    eb = np.asarray(g["expand_b"], f32)
    fb1 = np.asarray(g["ffn_b1"], f32)
    bns_full = np.asarray(g["bn_gamma"], f32) / np.sqrt(np.asarray(g["bn_var"], f32) + BN_EPS)
    bnb_full = np.asarray(g["bn_beta"], f32) - np.asarray(g["bn_mean"], f32) * bns_full
    s1 = np.asarray(g["sb1_scale"], f32)
    bb1 = np.asarray(g["proj_b"], f32) * s1 + np.asarray(g["sb1_bias"], f32)
    s2 = np.asarray(g["sb2_scale"], f32)
    b2full = np.asarray(g["ffn_b2"], f32) * s2 + np.asarray(g["sb2_bias"], f32)

    shared = {
        "w1": w1, "w2": w2, "w3": np.asarray(w3), "pw": np.asarray(pwa),
        "dwd": np.asarray(dwd), "band": np.asarray(band),
        "b1a": ptile(eb[:H], 6), "b1g": ptile(eb[H:], 6),
        "b2a": ptile(fb1[:H], 6), "b2g": ptile(fb1[H:], 6),
        "bns": ptile(bns_full, 6), "bnb": ptile(bnb_full, 6),
        "s1": ptile(s1, 3), "bb1": ptile(bb1, 3),
        "s2": ptile(s2, 3),
        "b2r": np.ascontiguousarray(b2full[None, :]).astype(bf16),
        "lng": ptile(np.asarray(g["ln_gamma"], f32), 3),
        "lnb": ptile(np.asarray(g["ln_beta"], f32), 3),
        "onesc": np.ones((128, 1), f32),
    }
    in_maps = []
    for core in range(NCORES):
        m = dict(shared)
        m["xt"] = np.ascontiguousarray(xt[core * BPC:(core + 1) * BPC])
        in_maps.append(m)
    return in_maps


def run(inputs, trace=False):
    from concourse.bass_utils import run_bass_kernel_spmd
    if "nc" not in _CACHE:
        _CACHE["nc"] = _build()
    nc = _CACHE["nc"]
    in_maps = _host_arrays(inputs)
    res = run_bass_kernel_spmd(nc, in_maps, list(range(NCORES)), trace=trace)
    parts = [res.results[c]["out"] for c in range(NCORES)]  # each [BPC, C, T]
    out = np.concatenate(parts, axis=0)                     # [B, C, T]
    return np.ascontiguousarray(out.transpose(0, 2, 1)), res


def kernel(**inputs):
    out, _ = run(inputs, trace=False)
    return out
